# revision 1
# baseline (speedup 1.0000x reference)
"""Int4-quantized column-parallel linear (LLaMA-7B FFN up-proj) on 8 TRN2 cores.

y[b,s,o] = sum_i x[b,s,i] * (unpack_int4(weight_q)[o,i] * scale[o]) + bias[o]

Strategy (per core, 1/8 of out_features = 1376):
  - int4 nibbles are exactly representable in fp16; matmul with integer-valued
    fp16 weights, apply scale/bias to the fp32 PSUM result at drain time.
  - x is rounded to fp16 (2^-12 relative) and the matmul accumulates in fp32
    PSUM, so the end-to-end error is ~1e-4 — far inside the 2e-2 gate — at
    full PE rate (1 cycle/row, vs 4 for native fp32 matmul).
  - weights are unpacked+transposed once into SBUF [in, feat] (moving side);
    x token-tiles are PE-transposed to [in, tok] (stationary side); PSUM out
    tile is [tok=128, feat=1376] (3 banks), drained with scale*psum+bias.
"""

from contextlib import ExitStack

import numpy as np

import concourse.bass as bass
import concourse.tile as tile
from concourse import bacc, mybir
from concourse.masks import make_identity

F32 = mybir.dt.float32
F16 = mybir.dt.float16
I32 = mybir.dt.int32

B, S, IN, OUT = 4, 2048, 4096, 11008
NCORES = 8
TOK = B * S
FEAT = OUT // NCORES

P = 128


def _feat_banks(feat):
    """Split feat into <=512 chunks (one PSUM bank each)."""
    out = []
    c0 = 0
    while c0 < feat:
        out.append((c0, min(512, feat - c0)))
        c0 += 512
    return out


def _feat_tiles(feat):
    out = []
    f0 = 0
    while f0 < feat:
        out.append((f0, min(P, feat - f0)))
        f0 += P
    return out


def build(tok=TOK, in_dim=IN, feat=FEAT):
    assert tok % P == 0 and in_dim % 256 == 0
    kp = in_dim // P       # number of 128-wide K tiles
    ntok = tok // P        # number of 128-row token tiles
    half = in_dim // 2
    banks = _feat_banks(feat)
    ftiles = _feat_tiles(feat)
    KGRP = 8                       # transposes per PSUM staging tile
    n_tg = (kp + KGRP - 1) // KGRP  # staging groups per token tile

    nc = bacc.Bacc("TRN2", target_bir_lowering=False, debug=False,
                   num_devices=NCORES)
    x_d = nc.dram_tensor("x", [tok, in_dim], F32, kind="ExternalInput").ap()
    wq_d = nc.dram_tensor("wq", [feat, half], I32, kind="ExternalInput").ap()
    sc_d = nc.dram_tensor("scale", [feat], F32, kind="ExternalInput").ap()
    bi_d = nc.dram_tensor("bias", [feat], F32, kind="ExternalInput").ap()
    y_d = nc.dram_tensor("y", [tok, feat], F32, kind="ExternalOutput").ap()

    with tile.TileContext(nc) as tc, ExitStack() as ctx:
        const = ctx.enter_context(tc.tile_pool(name="const", bufs=1))
        wtp = ctx.enter_context(tc.tile_pool(name="wt", bufs=1))
        in8k = ctx.enter_context(tc.tile_pool(name="in8k", bufs=4))
        x16p = ctx.enter_context(tc.tile_pool(name="x16", bufs=2))
        xtp = ctx.enter_context(tc.tile_pool(name="xt", bufs=2))
        outp = ctx.enter_context(tc.tile_pool(name="out", bufs=2))
        pstage = ctx.enter_context(tc.tile_pool(name="pstage", bufs=2, space="PSUM"))
        pout = ctx.enter_context(tc.tile_pool(name="pout", bufs=2, space="PSUM"))

        ident = const.tile([P, P], F16)
        make_identity(nc, ident[:])
        scale_b = const.tile([P, feat], F32)
        bias_b = const.tile([P, feat], F32)
        nc.sync.dma_start(
            out=scale_b[:],
            in_=bass.AP(tensor=sc_d.tensor, offset=sc_d.offset,
                        ap=[[0, P], sc_d.ap[0]]),
        )
        nc.sync.dma_start(
            out=bias_b[:],
            in_=bass.AP(tensor=bi_d.tensor, offset=bi_d.offset,
                        ap=[[0, P], bi_d.ap[0]]),
        )

        # Persistent dequantized+transposed weights: [in(part), k-major feat]
        wT = wtp.tile([P, kp * feat], F16)
        wTv = wT[:].rearrange("p (k f) -> p k f", k=kp)

        # ---- Phase W: unpack int4 -> fp16, transpose to [in, feat] ----
        for f0, fsz in ftiles:
            wq_t = in8k.tile([P, half], I32, tag="in8k")
            nc.sync.dma_start(out=wq_t[:fsz], in_=wq_d[f0:f0 + fsz])
            # biased nibbles: n ^ 8 maps the 2's-complement nibble to n+8
            n_lo = in8k.tile([P, half], I32, tag="in8k")
            nc.vector.tensor_scalar(
                out=n_lo[:fsz], in0=wq_t[:fsz], scalar1=15, scalar2=8,
                op0=mybir.AluOpType.bitwise_and, op1=mybir.AluOpType.bitwise_xor)
            n_hi = in8k.tile([P, half], I32, tag="in8k")
            nc.vector.tensor_scalar(
                out=n_hi[:fsz], in0=wq_t[:fsz], scalar1=4, scalar2=8,
                op0=mybir.AluOpType.logical_shift_right,
                op1=mybir.AluOpType.bitwise_xor)
            wb = in8k.tile([P, in_dim], F16, tag="in8k")
            wbv = wb[:fsz].rearrange("p (i two) -> p two i", two=2)
            # even input positions = low nibble, odd = high nibble
            nc.vector.tensor_scalar(
                out=wbv[:, 0], in0=n_lo[:fsz], scalar1=8, scalar2=None,
                op0=mybir.AluOpType.subtract)
            nc.vector.tensor_scalar(
                out=wbv[:, 1], in0=n_hi[:fsz], scalar1=8, scalar2=None,
                op0=mybir.AluOpType.subtract)
            for g in range(n_tg):
                glen = min(KGRP, kp - g * KGRP)
                st = pstage.tile([P, KGRP * P], F16)
                for j in range(glen):
                    kb = g * KGRP + j
                    nc.tensor.transpose(
                        out=st[:, j * P:j * P + fsz],
                        in_=wb[:fsz, kb * P:(kb + 1) * P],
                        identity=ident[:fsz, :fsz])
                stv = st[:].rearrange("p (j f) -> p j f", j=KGRP)
                # stage copy on ACT (reads PSUM fine) so DVE is free to run
                # the next tile's unpack in parallel
                nc.scalar.activation(
                    out=wTv[:, g * KGRP:g * KGRP + glen, f0:f0 + fsz],
                    in_=stv[:, :glen, :fsz],
                    func=mybir.ActivationFunctionType.Copy)

        # ---- Main loop: software-pipelined over token tiles ----
        # iteration i: load x(i), round to fp16, PE-transpose x(i) blocks
        # interleaved with the matmuls of token-tile i-1; drain i-1.
        state = {}

        def emit_load_round(i):
            x16 = x16p.tile([P, in_dim], F16)
            for h in range(2):
                xh = in8k.tile([P, half], F32, tag="in8k")
                nc.sync.dma_start(
                    out=xh[:], in_=x_d[i * P:(i + 1) * P, h * half:(h + 1) * half])
                hs = slice(h * half, (h + 1) * half)
                nc.scalar.activation(out=x16[:, hs], in_=xh[:],
                                     func=mybir.ActivationFunctionType.Copy)
            xt = xtp.tile([P, kp * P], F16)
            state[i] = xt
            return x16, xt

        def emit_tgroup(x16, xt, g):
            # x transposes ride the DMA xbar (2-byte dtype), on the ACT hwdge
            # queue so the SP copy queue never switches xbar mode.
            glen = min(KGRP, kp - g * KGRP)
            for j in range(glen):
                kb = g * KGRP + j
                nc.scalar.dma_start_transpose(
                    out=xt[:, kb * P:(kb + 1) * P],
                    in_=x16[:, kb * P:(kb + 1) * P])

        def emit_mm_group(i, po, ks):
            xt = state[i]
            for k in ks:
                lhsT = xt[:, k * P:(k + 1) * P]
                for c0, csz in banks:
                    nc.tensor.matmul(
                        out=po[:, c0:c0 + csz],
                        lhsT=lhsT,
                        rhs=wT[:, k * feat + c0:k * feat + c0 + csz],
                        start=(k == 0),
                        stop=(k == kp - 1))

        def emit_drain(i, po):
            ot = outp.tile([P, feat], F32)
            nc.vector.tensor_tensor(out=ot[:], in0=po[:], in1=scale_b[:],
                                    op=mybir.AluOpType.mult)
            nc.vector.tensor_tensor(out=ot[:], in0=ot[:], in1=bias_b[:],
                                    op=mybir.AluOpType.add)
            nc.sync.dma_start(out=y_d[i * P:(i + 1) * P, :], in_=ot[:])

        kchunks = np.array_split(np.arange(kp), n_tg)

        for i in range(ntok + 1):
            if i < ntok:
                x16, xt = emit_load_round(i)
            if i >= 1:
                po = pout.tile([P, feat], F32)
            for g in range(n_tg):
                if i < ntok:
                    emit_tgroup(x16, xt, g)
                if i >= 1:
                    emit_mm_group(i - 1, po, list(kchunks[g]))
            if i >= 1:
                emit_drain(i - 1, po)
                del state[i - 1]

    nc.compile()
    return nc


_CACHE = {}


def _get_program():
    if "nc" not in _CACHE:
        _CACHE["nc"] = build()
    return _CACHE["nc"]


def kernel(x, weight_q, scale, bias):
    from concourse.bass_utils import run_bass_kernel_spmd

    try:
        import jax

        jax.config.update("jax_compilation_cache_dir", "/root/problem/jax_cache")
        jax.config.update("jax_persistent_cache_min_compile_time_secs", 0)
    except Exception:
        pass

    nc = _get_program()
    xr = np.ascontiguousarray(np.asarray(x, dtype=np.float32).reshape(TOK, IN))
    wq = np.asarray(weight_q, dtype=np.int32)
    sc = np.asarray(scale, dtype=np.float32)
    bi = np.asarray(bias, dtype=np.float32)
    in_maps = []
    for c in range(NCORES):
        f0 = c * FEAT
        in_maps.append({
            "x": xr,
            "wq": np.ascontiguousarray(wq[f0:f0 + FEAT]),
            "scale": np.ascontiguousarray(sc[f0:f0 + FEAT]),
            "bias": np.ascontiguousarray(bi[f0:f0 + FEAT]),
        })
    res = run_bass_kernel_spmd(nc, in_maps, list(range(NCORES))).results
    y = np.concatenate([res[c]["y"] for c in range(NCORES)], axis=1)
    return y.reshape(B, S, OUT)



# revision 2
# speedup vs baseline: 1.5877x; 1.5877x over previous
"""Int4-quantized column-parallel linear (LLaMA-7B FFN up-proj) on 8 TRN2 cores.

y[b,s,o] = sum_i x[b,s,i] * (unpack_int4(weight_q)[o,i] * scale[o]) + bias[o]

Strategy (per core, 1/8 of out_features = 1376):
  - fp8 DoubleRow matmul at 0.5 cycles/row (2x the fp16 rate). The two
    DoubleRow slots carry a hi/lo residual split of x: hi = fp8(x),
    lo = fp8(x - hi), so one DoubleRow matmul computes (hi+lo)^T @ w with
    ~2^-8 effective precision on x (rel err ~8e-4 end to end).
  - weights are int4 in [-8,7], exactly representable in fp8e4; the rhs AP
    duplicates the same weight bytes across both DoubleRow slots with a
    stride-0 dimension, so weights are stored once.
  - x hi/lo bytes are written interleaved (hi at even, lo at odd offsets) so
    one fp16-typed xbar DMA transpose moves both planes at once; the
    transposed pair block is exactly the [K, 2, tok] stationary AP DoubleRow
    wants.
  - 4-deep software pipeline: loads (SP+Pool queues) -> hi (ACT) -> lo (DVE)
    -> pair transposes (SP) -> matmuls (PE, 2 iterations behind) -> drain
    (DVE, 1 behind PE) -> store (Pool). PE is the only near-saturated engine.
"""

from contextlib import ExitStack

import numpy as np

import concourse.bass as bass
import concourse.tile as tile
from concourse import bacc, mybir
from concourse.masks import make_identity

F32 = mybir.dt.float32
F16 = mybir.dt.float16
F8 = mybir.dt.float8e4
I32 = mybir.dt.int32

B, S, IN, OUT = 4, 2048, 4096, 11008
NCORES = 8
TOK = B * S
FEAT = OUT // NCORES

P = 128


def _chunks(total, step):
    out = []
    c0 = 0
    while c0 < total:
        out.append((c0, min(step, total - c0)))
        c0 += step
    return out


def build(tok=TOK, in_dim=IN, feat=FEAT):
    assert tok % P == 0 and in_dim % 256 == 0
    kp = in_dim // P       # number of 128-deep K tiles
    ntok = tok // P        # number of 128-row token tiles
    half = in_dim // 2
    ftiles = _chunks(feat, P)      # phase-W feature tiles
    mchunks = _chunks(feat, 256)   # matmul output chunks (moving free = 512)
    KGRP = 16                      # transposes per PSUM staging tile
    n_tg = (kp + KGRP - 1) // KGRP

    nc = bacc.Bacc("TRN2", target_bir_lowering=False, debug=False,
                   num_devices=NCORES)
    x_d = nc.dram_tensor("x", [tok, in_dim], F32, kind="ExternalInput").ap()
    wq_d = nc.dram_tensor("wq", [feat, half], I32, kind="ExternalInput").ap()
    sc_d = nc.dram_tensor("scale", [feat], F32, kind="ExternalInput").ap()
    bi_d = nc.dram_tensor("bias", [feat], F32, kind="ExternalInput").ap()
    y_d = nc.dram_tensor("y", [tok, feat], F32, kind="ExternalOutput").ap()

    with tile.TileContext(nc) as tc, ExitStack() as ctx:
        const = ctx.enter_context(tc.tile_pool(name="const", bufs=1))
        wtp = ctx.enter_context(tc.tile_pool(name="wt", bufs=1))
        wscr = ctx.enter_context(tc.tile_pool(name="wscr", bufs=4))
        x32p = ctx.enter_context(tc.tile_pool(name="x32p", bufs=3))
        xpairp = ctx.enter_context(tc.tile_pool(name="xpairp", bufs=3))
        xtp = ctx.enter_context(tc.tile_pool(name="xtp", bufs=3))
        outp = ctx.enter_context(tc.tile_pool(name="outp", bufs=2))
        pstage = ctx.enter_context(tc.tile_pool(name="pstage", bufs=2, space="PSUM"))
        pout = ctx.enter_context(tc.tile_pool(name="pout", bufs=2, space="PSUM"))

        ident = const.tile([P, P], F8)
        make_identity(nc, ident[:])
        scale_b = const.tile([P, feat], F32)
        bias_b = const.tile([P, feat], F32)
        nc.sync.dma_start(
            out=scale_b[:],
            in_=bass.AP(tensor=sc_d.tensor, offset=sc_d.offset,
                        ap=[[0, P], sc_d.ap[0]]),
        )
        nc.sync.dma_start(
            out=bias_b[:],
            in_=bass.AP(tensor=bi_d.tensor, offset=bi_d.offset,
                        ap=[[0, P], bi_d.ap[0]]),
        )

        # Persistent fp8 weights, transposed: [in(part), k-major feat]
        wT = wtp.tile([P, kp * feat], F8)
        wTv = wT[:].rearrange("p (k f) -> p k f", k=kp)

        # ---- Phase W: unpack int4 -> fp8, transpose to [in, feat] ----
        def emit_phase_w():
            for f0, fsz in ftiles:
                wq_t = wscr.tile([P, half], I32, tag="w")
                nc.sync.dma_start(out=wq_t[:fsz], in_=wq_d[f0:f0 + fsz])
                wb = wscr.tile([P, in_dim], F8, tag="w")
                wbv = wb[:fsz].rearrange("p (i two) -> p two i", two=2)
                # sign-extend packed nibbles via shift pairs, convert to fp8
                nc.vector.tensor_scalar(
                    out=wbv[:, 0], in0=wq_t[:fsz], scalar1=28, scalar2=28,
                    op0=mybir.AluOpType.logical_shift_left,
                    op1=mybir.AluOpType.arith_shift_right)
                nc.vector.tensor_scalar(
                    out=wbv[:, 1], in0=wq_t[:fsz], scalar1=24, scalar2=28,
                    op0=mybir.AluOpType.logical_shift_left,
                    op1=mybir.AluOpType.arith_shift_right)
                for g in range(n_tg):
                    glen = min(KGRP, kp - g * KGRP)
                    st = pstage.tile([P, KGRP * P], F8)
                    for j in range(glen):
                        kb = g * KGRP + j
                        nc.tensor.transpose(
                            out=st[:, j * P:j * P + fsz],
                            in_=wb[:fsz, kb * P:(kb + 1) * P],
                            identity=ident[:fsz, :fsz])
                    stv = st[:].rearrange("p (j f) -> p j f", j=KGRP)
                    # stage copy on ACT (reads PSUM fine) so DVE stays free
                    nc.scalar.activation(
                        out=wTv[:, g * KGRP:g * KGRP + glen, f0:f0 + fsz],
                        in_=stv[:, :glen, :fsz],
                        func=mybir.ActivationFunctionType.Copy)

        emit_phase_w()

        # ---- Main loop: 4-deep software pipeline over token tiles ----
        x32s, xpairs, xts, pos, ots = {}, {}, {}, {}, {}

        def emit_load(i):
            x32 = x32p.tile([P, in_dim], F32)
            x32s[i] = x32
            nc.sync.dma_start(out=x32[:, :half],
                              in_=x_d[i * P:(i + 1) * P, :half])
            nc.gpsimd.dma_start(out=x32[:, half:],
                                in_=x_d[i * P:(i + 1) * P, half:])

        def emit_hi(i):
            x32 = x32s[i]
            xpair = xpairp.tile([P, 2 * in_dim], F8)
            xpairs[i] = xpair
            xpv = xpair[:].rearrange("p (i two) -> p two i", two=2)
            nc.scalar.activation(out=xpv[:, 0], in_=x32[:],
                                 func=mybir.ActivationFunctionType.Copy)

        def emit_lo(i):
            x32 = x32s[i]
            xpv = xpairs[i][:].rearrange("p (i two) -> p two i", two=2)
            nc.vector.tensor_tensor(out=xpv[:, 1], in0=x32[:], in1=xpv[:, 0],
                                    op=mybir.AluOpType.subtract)

        def emit_transposes(i):
            xpair = xpairs[i]
            xt = xtp.tile([P, kp * 2 * P], F8)
            xts[i] = xt
            for k in range(kp):
                nc.sync.dma_start_transpose(
                    out=xt[:, k * 2 * P:(k + 1) * 2 * P].bitcast(F16),
                    in_=xpair[:, k * 2 * P:(k + 1) * 2 * P].bitcast(F16))
            del xpairs[i]

        def emit_matmuls(i):
            xt = xts[i]
            po = pout.tile([P, feat], F32)
            pos[i] = po
            for k in range(kp):
                lhsT = bass.AP(
                    tensor=xt.tensor, offset=xt[:].offset + k * 2 * P,
                    ap=[xt[:].ap[0], [1, 2], [2, P]])
                for ci, (c0, csz) in enumerate(mchunks):
                    first_in_bank = c0 % 512 == 0
                    last_in_bank = (ci == len(mchunks) - 1
                                    or mchunks[ci + 1][0] % 512 == 0)
                    rhs = bass.AP(
                        tensor=wT.tensor, offset=wT[:].offset + k * feat + c0,
                        ap=[wT[:].ap[0], [0, 2], [1, csz]])
                    nc.tensor.matmul(
                        out=po[:, c0:c0 + csz], lhsT=lhsT, rhs=rhs,
                        start=(k == 0 and first_in_bank),
                        stop=(k == kp - 1 and last_in_bank),
                        perf_mode=mybir.MatmulPerfMode.DoubleRow)
            del x32s[i], xts[i]

        def emit_drain(i):
            po = pos[i]
            ot = outp.tile([P, feat], F32)
            ots[i] = ot
            nc.vector.tensor_tensor(out=ot[:], in0=po[:], in1=scale_b[:],
                                    op=mybir.AluOpType.mult)
            nc.vector.tensor_tensor(out=ot[:], in0=ot[:], in1=bias_b[:],
                                    op=mybir.AluOpType.add)
            del pos[i]

        def emit_store(i):
            nc.gpsimd.dma_start(out=y_d[i * P:(i + 1) * P, :], in_=ots[i][:])
            del ots[i]

        for i in range(ntok + 4):
            if i < ntok:
                emit_load(i)
            if 1 <= i <= ntok:
                emit_hi(i - 1)
                emit_lo(i - 1)
            if 4 <= i:
                emit_drain(i - 4)
            if 1 <= i <= ntok:
                emit_transposes(i - 1)
            if 3 <= i < ntok + 3:
                emit_matmuls(i - 3)
            if 4 <= i:
                emit_store(i - 4)

    nc.compile()
    return nc


_CACHE = {}


def _get_program():
    if "nc" not in _CACHE:
        _CACHE["nc"] = build()
    return _CACHE["nc"]


def kernel(x, weight_q, scale, bias):
    from concourse.bass_utils import run_bass_kernel_spmd

    try:
        import jax

        jax.config.update("jax_compilation_cache_dir", "/root/problem/jax_cache")
        jax.config.update("jax_persistent_cache_min_compile_time_secs", 0)
    except Exception:
        pass

    nc = _get_program()
    xr = np.ascontiguousarray(np.asarray(x, dtype=np.float32).reshape(TOK, IN))
    wq = np.asarray(weight_q, dtype=np.int32)
    sc = np.asarray(scale, dtype=np.float32)
    bi = np.asarray(bias, dtype=np.float32)
    in_maps = []
    for c in range(NCORES):
        f0 = c * FEAT
        in_maps.append({
            "x": xr,
            "wq": np.ascontiguousarray(wq[f0:f0 + FEAT]),
            "scale": np.ascontiguousarray(sc[f0:f0 + FEAT]),
            "bias": np.ascontiguousarray(bi[f0:f0 + FEAT]),
        })
    res = run_bass_kernel_spmd(nc, in_maps, list(range(NCORES))).results
    y = np.concatenate([res[c]["y"] for c in range(NCORES)], axis=1)
    return y.reshape(B, S, OUT)
